# revision 5
# baseline (speedup 1.0000x reference)
"""BottleneckAdapter kernel for Trainium2 (Bass/Tile), 8-way data parallel.

out = x + scale * (gelu(LN(x) @ w_down + b_down) @ w_up + b_up)

v2 strategy per core (2048 tokens, 16 tiles of [128, 1024], weights
replicated). The v1 kernel was DMA-bound: the DMA-xbar transposes added
4MB of SBUF<->SBUF traffic to the 16MB of HBM traffic on the same DMA
engines AND forced a serialized load->transpose->store window (93.4us).
v2 moves the transpose to the tensor engine and keeps every engine's
per-tile work under the per-tile DMA time (~2.9us):

  - DMA (gpsimd SWDGE queue): ONLY the 16 per-tile 512KB loads and 16
    stores. No SBUF<->SBUF traffic, no xbar mode switches, loads all
    issued up-front so the pipeline is never input-starved.
  - ACT: fp32->bf16 cast (accum_out -> sum(x)), x^2 via AF.Square
    (accum_out -> sum(x^2)), one PSUM->SBUF evac half per tile, gelu.
  - PE: 8 [128,128] bf16 transposes per tile (identity matmul) into
    PSUM, 8 down-proj matmuls (z = x@W', W' = norm_w*w_down), the small
    zh transpose, 2 up-proj matmuls ([65,128]x[65,512] with a ones row
    adding scale*b_up).
  - DVE: per-quarter LN stats smalls, the z corrections
    (zh = rstd*z + (mu*rstd)*(-colsum W') + b'), the other evac half,
    and the exact fp32 residual out = u + x straight from PSUM.
  - LN folded into the down-projection as in v1:
        LN(x) @ W' = rstd * (x @ W') - (rstd*mu) * colsum(W') + b'.
"""

import numpy as np

import concourse.bass as bass
import concourse.bacc as bacc
import concourse.mybir as mybir
import concourse.tile as tile
from concourse import bass_utils
from concourse.masks import make_identity

F32 = mybir.dt.float32
BF16 = mybir.dt.bfloat16
AF = mybir.ActivationFunctionType
OP = mybir.AluOpType

# Problem shapes (hardcoded per the contract).
B, N, D = 4, 4096, 1024
BN = 64                      # bottleneck
N_CORES = 8
TOK_TOTAL = B * N            # 16384
TOK = TOK_TOTAL // N_CORES   # 2048 tokens per core
P = 128                      # partitions
NT = TOK // P                # 16 token tiles per core
TPQ = 4                      # tiles per stats quarter
NQ = NT // TPQ
NCH = D // P                 # 8 contraction chunks of 128
EPS = 1e-5
H = D // 2                   # 512 (psum bank half)
HC = NCH // 2                # 4 chunks per transpose psum tile


def _build_kernel():
    nc = bacc.Bacc(
        "TRN2",
        target_bir_lowering=False,
        debug=False,
        enable_asserts=False,
        num_devices=N_CORES,
    )
    x_d = nc.dram_tensor("x", [TOK, D], F32, kind="ExternalInput")
    nw_d = nc.dram_tensor("norm_w", [D], F32, kind="ExternalInput")
    nb_d = nc.dram_tensor("norm_b", [D], F32, kind="ExternalInput")
    wd_d = nc.dram_tensor("w_down", [D, BN], F32, kind="ExternalInput")
    bd_d = nc.dram_tensor("b_down", [BN], F32, kind="ExternalInput")
    wu_d = nc.dram_tensor("w_up", [BN, D], F32, kind="ExternalInput")
    bu_d = nc.dram_tensor("b_up", [D], F32, kind="ExternalInput")
    sc_d = nc.dram_tensor("scale", [1, 1], F32, kind="ExternalInput")
    out_d = nc.dram_tensor("out", [TOK, D], F32, kind="ExternalOutput")

    with tile.TileContext(nc) as tc:
        _body(
            tc,
            x_d.ap(),
            nw_d.ap(),
            nb_d.ap(),
            wd_d.ap(),
            bd_d.ap(),
            wu_d.ap(),
            bu_d.ap(),
            sc_d.ap(),
            out_d.ap(),
        )
    nc.compile()
    return nc


def _body(tc, x, nw, nb, wd, bd, wu, bu, sc, out):
    from contextlib import ExitStack

    nc = tc.nc
    ctx = ExitStack()
    with ctx:
        x_r = x.rearrange("(t p) d -> p t d", p=P)      # [128, 16, 1024]
        out_r = out.rearrange("(t p) d -> p t d", p=P)

        const = ctx.enter_context(tc.tile_pool(name="const", bufs=1))
        px = ctx.enter_context(tc.tile_pool(name="px", bufs=NT))      # x f32, full residency

        # ---------- x loads first: all 16 tiles issued immediately ----------
        xqs = []
        for t in range(NT):
            xq = px.tile([P, D], F32, tag="xq")
            nc.gpsimd.dma_start(out=xq, in_=x_r[:, t, :])
            xqs.append(xq)

        # ---------- constants / preprocessing ----------
        eps_b = const.tile([P, 1], F32)
        nc.vector.memset(eps_b, EPS)
        ones_col = const.tile([P, 1], BF16)
        nc.vector.memset(ones_col, 1.0)
        one_1 = const.tile([1, 1], BF16)
        nc.vector.memset(one_1, 1.0)
        ones_row = const.tile([1, P], BF16)
        nc.vector.memset(ones_row, 1.0)

        # W' = norm_w[:,None] * w_down laid out [p, c, j]; kept fp32 + bf16.
        w_f32 = const.tile([P, NCH, BN], F32)
        nc.sync.dma_start(out=w_f32, in_=wd.rearrange("(c p) j -> p c j", p=P))
        nw_sb = const.tile([P, NCH], F32)
        nc.sync.dma_start(out=nw_sb, in_=nw.rearrange("(c p) -> p c", p=P))
        w_sb = const.tile([P, NCH, BN], BF16)
        for c in range(NCH):
            nc.vector.tensor_scalar_mul(
                w_sb[:, c, :], w_f32[:, c, :], nw_sb[:, c : c + 1]
            )

        ident_bf = const.tile([P, P], BF16)
        make_identity(nc, ident_bf)

        nb_sb = const.tile([P, NCH, 1], F32)
        nc.sync.dma_start(out=nb_sb[:, :, 0], in_=nb.rearrange("(c p) -> p c", p=P))
        bd_f = const.tile([1, BN], F32)
        nc.sync.dma_start(out=bd_f, in_=bd[None, :])
        bd_sb = const.tile([1, BN], BF16)
        nc.vector.tensor_scalar_mul(bd_sb, bd_f, 1.0)

        # w_up_ext = scale * [w_up; b_up]  -> bf16 [65, 1024]
        wue_f = const.tile([BN + 1, D], F32)
        nc.sync.dma_start(out=wue_f[0:BN, :], in_=wu)
        nc.sync.dma_start(out=wue_f[BN : BN + 1, :], in_=bu[None, :])
        sc_b = const.tile([BN + 1, 1], F32)
        nc.sync.dma_start(
            out=sc_b,
            in_=bass.AP(tensor=sc.tensor, offset=0, ap=[[0, BN + 1], [1, 1]]),
        )
        wue = const.tile([BN + 1, D], BF16)
        nc.vector.tensor_scalar_mul(wue, wue_f, sc_b)

        # ---------- pools ----------
        pxb = ctx.enter_context(tc.tile_pool(name="pxb", bufs=3))     # x bf16 tiles
        pxt = ctx.enter_context(tc.tile_pool(name="pxt", bufs=3))     # xT tiles
        psq = ctx.enter_context(tc.tile_pool(name="psq", bufs=2))     # x^2 scratch
        pst = ctx.enter_context(tc.tile_pool(name="pst", bufs=2))     # per-quarter stats
        psc = ctx.enter_context(tc.tile_pool(name="psc", bufs=2))     # z-corr temps
        pgt = ctx.enter_context(tc.tile_pool(name="pgt", bufs=3))     # gT tiles
        pout = ctx.enter_context(tc.tile_pool(name="pout", bufs=3))   # out staging
        tps = ctx.enter_context(tc.tile_pool(name="tps", bufs=3, space="PSUM"))
        zps = ctx.enter_context(tc.tile_pool(name="zps", bufs=2, space="PSUM"))
        ztps = ctx.enter_context(tc.tile_pool(name="ztps", bufs=1, space="PSUM"))
        ups = ctx.enter_context(tc.tile_pool(name="ups", bufs=2, space="PSUM"))

        def preproc_rows():
            """s = -colsum(W'); b' = b_down + norm_b @ w_down; broadcast both
            across partitions via K=1 matmuls (PSUM via zps pool)."""
            s_ps = zps.tile([1, BN], F32, tag="z")
            for c in range(NCH):
                nc.tensor.matmul(
                    s_ps, ones_col, w_sb[:, c, :], start=(c == 0), stop=(c == NCH - 1)
                )
            s_neg = const.tile([1, BN], BF16)
            nc.scalar.mul(s_neg, s_ps, -1.0)
            rep_ps = zps.tile([P, BN], F32, tag="z")
            nc.tensor.matmul(rep_ps, ones_row, s_neg, start=True, stop=True)
            sneg_r = const.tile([P, BN], F32)
            nc.scalar.copy(sneg_r, rep_ps)

            bp_ps = zps.tile([1, BN], F32, tag="z")
            for c in range(NCH):
                nc.tensor.matmul(
                    bp_ps, nb_sb[:, c, :], w_f32[:, c, :], start=(c == 0), stop=False
                )
            nc.tensor.matmul(bp_ps, one_1, bd_sb, start=False, stop=True)
            b_row = const.tile([1, BN], BF16)
            nc.scalar.copy(b_row, bp_ps)
            rep_ps2 = zps.tile([P, BN], F32, tag="z")
            nc.tensor.matmul(rep_ps2, ones_row, b_row, start=True, stop=True)
            b_rep = const.tile([P, BN], F32)
            nc.scalar.copy(b_rep, rep_ps2)
            return sneg_r, b_rep

        sneg_r, b_rep = preproc_rows()

        # ---------- per-tile pipeline stages ----------
        st = {}      # per-quarter stats tiles
        xbs = {}     # bf16 tiles
        xts = {}     # transposed tiles
        gts = {}     # gelu output tiles

        def emit_A(t):
            """cast (ACT, accum->sumx) + square (ACT, accum->sumsq)."""
            q, r = divmod(t, TPQ)
            if r == 0:
                st[q] = {
                    "sumx": pst.tile([P, TPQ], F32, tag="sumx", name="sumx"),
                    "sumsq": pst.tile([P, TPQ], F32, tag="sumsq", name="sumsq"),
                }
            xb = pxb.tile([P, D], BF16, tag="xb")
            nc.scalar.activation(
                xb, xqs[t], AF.Copy, accum_out=st[q]["sumx"][:, r : r + 1]
            )
            xbs[t] = xb
            x2 = psq.tile([P, D], BF16, tag="x2")
            nc.scalar.activation(
                x2, xqs[t], AF.Square, accum_out=st[q]["sumsq"][:, r : r + 1]
            )
            if r == TPQ - 1:
                emit_stats(q)

        def emit_stats(q):
            """mu = sumx/D ; var = sumsq/D - mu^2 ; rstd = 1/sqrt(var+eps)."""
            s = st[q]
            mu = pst.tile([P, TPQ], F32, tag="mu")
            nc.vector.tensor_scalar_mul(mu, s["sumx"], 1.0 / D)
            musq = pst.tile([P, TPQ], F32, tag="musq")
            nc.vector.tensor_mul(musq, mu, mu)
            var = pst.tile([P, TPQ], F32, tag="var")
            nc.vector.scalar_tensor_tensor(
                out=var, in0=s["sumsq"], scalar=1.0 / D, in1=musq,
                op0=OP.mult, op1=OP.subtract,
            )
            srt = pst.tile([P, TPQ], F32, tag="srt")
            nc.scalar.activation(srt, var, AF.Sqrt, bias=eps_b)
            rstd = pst.tile([P, TPQ], F32, tag="rstd")
            nc.vector.reciprocal(rstd, srt)
            mr = pst.tile([P, TPQ], F32, tag="mr")
            nc.vector.tensor_mul(mr, mu, rstd)
            s["rstd"] = rstd
            s["mr"] = mr

        def emit_B(t):
            """PE transposes (bf16 identity matmuls) + PSUM->SBUF evac."""
            xb = xbs.pop(t)
            xt = pxt.tile([P, NCH, P], BF16, tag="xt")
            for half in range(2):
                ps = tps.tile([P, HC, P], BF16, tag="tp")
                for j in range(HC):
                    c = half * HC + j
                    nc.tensor.transpose(
                        ps[:, j, :], xb[:, c * P : (c + 1) * P], ident_bf
                    )
                lo, hi = half * HC, (half + 1) * HC
                if half == 0:
                    nc.scalar.copy(xt[:, lo:hi, :], ps)
                else:
                    nc.vector.tensor_scalar_mul(xt[:, lo:hi, :], ps, 1.0)
            xts[t] = xt

        def emit_C(t):
            """down matmuls + z corrections + zh transpose + gelu."""
            q, r = divmod(t, TPQ)
            xt = xts.pop(t)
            z = zps.tile([P, BN], F32, tag="z")
            for c in range(NCH):
                nc.tensor.matmul(
                    z, xt[:, c, :], w_sb[:, c, :],
                    start=(c == 0), stop=(c == NCH - 1),
                )
            rstd = st[q]["rstd"][:, r : r + 1]
            mr = st[q]["mr"][:, r : r + 1]
            t3 = psc.tile([P, BN], F32, tag="t3")
            nc.vector.scalar_tensor_tensor(
                out=t3, in0=sneg_r, scalar=mr, in1=b_rep, op0=OP.mult, op1=OP.add
            )
            zh = psc.tile([P, BN], BF16, tag="zh")
            nc.vector.scalar_tensor_tensor(
                out=zh, in0=z, scalar=rstd, in1=t3, op0=OP.mult, op1=OP.add
            )
            zt = ztps.tile([BN, P], BF16, tag="zt")
            nc.tensor.transpose(zt, zh, ident_bf)
            gt = pgt.tile([BN + 1, P], BF16, tag="gt")
            nc.vector.memset(gt[BN : BN + 1, :], 1.0)
            nc.scalar.activation(gt[0:BN, :], zt, AF.Gelu)
            gts[t] = gt

        def emit_D(t):
            """up matmuls + exact fp32 residual + store."""
            gt = gts.pop(t)
            ot = pout.tile([P, D], F32, tag="ot")
            for h in range(2):
                u = ups.tile([P, H], F32, tag="u")
                nc.tensor.matmul(
                    u, gt, wue[:, h * H : (h + 1) * H], start=True, stop=True
                )
                nc.vector.scalar_tensor_tensor(
                    out=ot[:, h * H : (h + 1) * H],
                    in0=u,
                    scalar=1.0,
                    in1=xqs[t][:, h * H : (h + 1) * H],
                    op0=OP.mult,
                    op1=OP.add,
                )
            nc.gpsimd.dma_start(out=out_r[:, t, :], in_=ot)

        # Skewed emission: per-engine queues stay per-tile round-robin with
        # enough lookahead that no queue head-blocks on a later stage.
        for step in range(NT + 4):
            if step < NT:
                emit_A(step)
            if 0 <= step - 2 < NT:
                emit_B(step - 2)
            if 0 <= step - 3 < NT:
                emit_C(step - 3)
            if 0 <= step - 4 < NT:
                emit_D(step - 4)


_NC = None


def _get_nc():
    global _NC
    if _NC is None:
        _NC = _build_kernel()
    return _NC


def _make_in_maps(inputs):
    x = np.ascontiguousarray(np.asarray(inputs["x"], dtype=np.float32)).reshape(
        TOK_TOTAL, D
    )
    shared = {
        "norm_w": np.ascontiguousarray(np.asarray(inputs["norm_w"], np.float32)),
        "norm_b": np.ascontiguousarray(np.asarray(inputs["norm_b"], np.float32)),
        "w_down": np.ascontiguousarray(np.asarray(inputs["w_down"], np.float32)),
        "b_down": np.ascontiguousarray(np.asarray(inputs["b_down"], np.float32)),
        "w_up": np.ascontiguousarray(np.asarray(inputs["w_up"], np.float32)),
        "b_up": np.ascontiguousarray(np.asarray(inputs["b_up"], np.float32)),
        "scale": np.asarray(inputs["scale"], np.float32).reshape(1, 1),
    }
    in_maps = []
    for c in range(N_CORES):
        m = dict(shared)
        m["x"] = np.ascontiguousarray(x[c * TOK : (c + 1) * TOK])
        in_maps.append(m)
    return in_maps


def run(inputs, trace=False, **kwargs):
    nc = _get_nc()
    in_maps = _make_in_maps(inputs)
    res = bass_utils.run_bass_kernel_spmd(
        nc, in_maps, core_ids=list(range(N_CORES)), trace=trace, **kwargs
    )
    shards = [res.results[c]["out"] for c in range(N_CORES)]
    full = np.concatenate(shards, axis=0).reshape(B, N, D).astype(np.float32)
    return full, res


def kernel(**inputs):
    full, _ = run(inputs, trace=False)
    return full
